# revision 13
# baseline (speedup 1.0000x reference)
"""Trainium2 Bass kernel for Performer/FAVOR+ linear attention over nodes.

Computes (out, q_prime, k_prime) matching the reference:
    q' = ratio*(exp(dd_q - diag_q - rowmax(dd_q)) + eps)        [N, M]
    k' = ratio*(exp(dd_k - diag_k - globalmax(dd_k)) + eps)     [N, M]
    kv = einsum("nm,bnc->bmc", k', x)                           [B, M, C]
    num = einsum("nm,bmc->bnc", q', kv)                         [B, N, C]
    denom = q' @ sum_n k'                                       [N]
    out = num / denom[None, :, None]                            [B, N, C]

Sharding: pure data-parallel over batch B across 8 cores (2 batches per
core, fused side-by-side in the matmul free dimension).  Feature maps are
computed redundantly on every core; no collectives.

The global max for k' is handled by storing e_k = exp(dd_k - diag_k) and
post-scaling by exp(-stab) once stab is known (exact up to fp rounding).
"""

import math

import numpy as np

import concourse.bass as bass
from concourse import bacc
import concourse.mybir as mybir
import concourse.tile as tile
from concourse.masks import make_identity

# Problem constants (hardcoded per contract; kernel.py must be self-contained).
B, N, C, D, M = 16, 20000, 256, 64, 64
NCORES = 8
BPC = B // NCORES  # batches per core
P = 128
TAU = 1.0
EPS = 1e-4
SCALE = (1.0 / math.sqrt(TAU)) * (D ** -0.25)  # node_vec -> dn multiplier
RATIO = 1.0 / math.sqrt(M)

F32 = mybir.dt.float32
AX = mybir.AxisListType
ALU = mybir.AluOpType
ACTF = mybir.ActivationFunctionType


def _flat(ap):
    """View a [p, a, b] AP as [p, a*b]."""
    if len(ap.shape) == 2:
        return ap
    return ap.rearrange("p a b -> p (a b)")


def build(n_nodes=N, bpc=BPC):
    """Build the single-core Bass program (SPMD across cores)."""
    nt = (n_nodes + P - 1) // P
    nc = bacc.Bacc()

    x2 = nc.declare_dram_parameter("x2", [bpc, n_nodes, C], F32, isOutput=False)
    nv1 = nc.declare_dram_parameter("nv1", [n_nodes, D], F32, isOutput=False)
    nv2 = nc.declare_dram_parameter("nv2", [n_nodes, D], F32, isOutput=False)
    proj = nc.declare_dram_parameter("proj", [M, D], F32, isOutput=False)
    out2 = nc.declare_dram_parameter("out2", [bpc, n_nodes, C], F32, isOutput=True)
    qp = nc.declare_dram_parameter("qp", [n_nodes, M], F32, isOutput=True)
    kp = nc.declare_dram_parameter("kp", [n_nodes, M], F32, isOutput=True)

    with tile.TileContext(nc) as tc:
        with (
            tc.tile_pool(name="singles", bufs=1) as singles,
            tc.tile_pool(name="work", bufs=3) as work,
            tc.tile_pool(name="xin", bufs=4) as xin,
            tc.tile_pool(name="outp", bufs=4) as outp,
        ):
            # --- constants ---
            ident = singles.tile([P, P], F32)
            make_identity(nc, ident)
            ones_col = singles.tile([P, 1], F32)
            nc.vector.memset(ones_col, 1.0)
            ones_row = singles.tile([1, P], F32)
            nc.vector.memset(ones_row, 1.0)

            # resident buffers
            kbuf = singles.tile([P, nt * M], F32)      # k' tiles, [n, m] layout
            qT_buf = singles.tile([M, nt * P], F32)    # q'^T, [m, n] layout
            tmk_all = singles.tile([P, nt], F32)       # per-tile dd_k maxes
            nc.vector.memset(tmk_all, -1e30)

            # proj^T, pre-scaled by SCALE
            proj_sb = singles.tile([M, D], F32)
            nc.sync.dma_start(out=proj_sb, in_=proj[:, :])
            projT_s = singles.tile([D, M], F32)

            # ================= Phase A: feature maps =================
            with tc.tile_pool(name="ps_feat", bufs=2, space="PSUM") as ps_feat:
                # Prime PE's vector-clock view of the identity (Pool engine)
                # so later transpose matmuls carry at most one sem wait (the
                # single-instruction transpose form has one HW wait slot).
                prime_ps = ps_feat.tile([1, P], F32, tag="qTt")
                nc.tensor.transpose(out=prime_ps, in_=ident[:, 0:1], identity=ident)

                pT_ps = ps_feat.tile([D, M], F32, tag="nvT12")
                nc.tensor.transpose(out=pT_ps, in_=proj_sb, identity=ident[:M, :M])
                nc.vector.tensor_scalar_mul(out=projT_s, in0=pT_ps, scalar1=SCALE)

                for i in range(nt):
                    n0 = i * P
                    pr = min(P, n_nodes - n0)

                    nv12 = work.tile([P, 2, D], F32)
                    nc.sync.dma_start(out=nv12[:pr, 0, :], in_=nv1[n0 : n0 + pr, :])
                    nc.sync.dma_start(out=nv12[:pr, 1, :], in_=nv2[n0 : n0 + pr, :])

                    # two transposes (one per nv) so each carries a single DMA
                    # wait, side by side in the free dim of one PSUM bank
                    # (transpose outputs must start at PSUM partition 0)
                    nvT_ps = ps_feat.tile([D, 2, P], F32, tag="nvT12")
                    nc.tensor.transpose(
                        out=nvT_ps[:, 0, :pr], in_=nv12[:pr, 0, :],
                        identity=ident[:pr, :pr],
                    )
                    nc.tensor.transpose(
                        out=nvT_ps[:, 1, :pr], in_=nv12[:pr, 1, :],
                        identity=ident[:pr, :pr],
                    )
                    nvT = work.tile([D, 2, P], F32)
                    nc.vector.tensor_copy(
                        out=nvT[:, :, :pr], in_=nvT_ps[:, :, :pr]
                    )

                    # dd = (s*nv) @ proj^T for both vecs (both operands base 0)
                    ddq_ps = ps_feat.tile([P, M], F32, tag="ddq")
                    ddk_ps = ps_feat.tile([P, M], F32, tag="ddk")
                    nc.tensor.matmul(
                        out=ddq_ps[:pr], lhsT=nvT[:, 0, :pr], rhs=projT_s,
                        start=True, stop=True,
                    )
                    nc.tensor.matmul(
                        out=ddk_ps[:pr], lhsT=nvT[:, 1, :pr], rhs=projT_s,
                        start=True, stop=True,
                    )

                    # diag: sum of nv^2 over d, for both vecs at once
                    sq = work.tile([P, 2, D], F32)
                    nc.vector.tensor_mul(sq[:pr], nv12[:pr], nv12[:pr])
                    diag12 = work.tile([P, 2], F32)
                    nc.vector.tensor_reduce(
                        out=diag12[:pr], in_=sq[:pr], axis=AX.X, op=ALU.add
                    )

                    # q: per-row max stabilizer
                    rmq = work.tile([P, 1], F32)
                    nc.vector.tensor_reduce(
                        out=rmq[:pr], in_=ddq_ps[:pr], axis=AX.X, op=ALU.max
                    )
                    negbq = work.tile([P, 1], F32)
                    nc.vector.tensor_scalar(
                        out=negbq[:pr], in0=diag12[:pr, 0:1],
                        scalar1=-0.5 * SCALE * SCALE, scalar2=rmq[:pr],
                        op0=ALU.mult, op1=ALU.subtract,
                    )
                    # k: per-tile max (global resolved later)
                    nc.vector.tensor_reduce(
                        out=tmk_all[:pr, i : i + 1], in_=ddk_ps[:pr], axis=AX.X, op=ALU.max
                    )
                    negbk = work.tile([P, 1], F32)
                    nc.vector.tensor_scalar_mul(
                        out=negbk[:pr], in0=diag12[:pr, 1:2], scalar1=-0.5 * SCALE * SCALE
                    )

                    # q' tile
                    expq = work.tile([P, M], F32)
                    nc.scalar.activation(
                        out=expq[:pr], in_=ddq_ps[:pr], func=ACTF.Exp, bias=negbq[:pr]
                    )
                    qtile = work.tile([P, M], F32)
                    nc.vector.tensor_scalar(
                        out=qtile[:pr], in0=expq[:pr],
                        scalar1=RATIO, scalar2=RATIO * EPS,
                        op0=ALU.mult, op1=ALU.add,
                    )
                    nc.sync.dma_start(out=qp[n0 : n0 + pr, :], in_=qtile[:pr])

                    # q'^T into resident buffer
                    qT_ps = ps_feat.tile([M, P], F32, tag="qTt")
                    nc.tensor.transpose(
                        out=qT_ps[:, :pr], in_=qtile[:pr], identity=ident[:pr, :pr]
                    )
                    nc.vector.tensor_copy(
                        out=qT_buf[:, n0 : n0 + pr], in_=qT_ps[:, :pr]
                    )

                    # e_k = exp(dd_k - diag_k), stabilizer applied in pass 2
                    nc.scalar.activation(
                        out=kbuf[:pr, i * M : (i + 1) * M], in_=ddk_ps[:pr],
                        func=ACTF.Exp, bias=negbk[:pr],
                    )

                # ---- pass 2: resolve global k stabilizer, finalize k' ----
                kmax_col = work.tile([P, 1], F32)
                nc.vector.tensor_reduce(
                    out=kmax_col, in_=tmk_all[:, :nt], axis=AX.X, op=ALU.max
                )
                km_ps = ps_feat.tile([1, P], F32, tag="nvT12")
                nc.tensor.transpose(out=km_ps, in_=kmax_col, identity=ident)
                kst = work.tile([1, 1], F32)
                nc.vector.tensor_reduce(out=kst, in_=km_ps, axis=AX.X, op=ALU.max)
                bc_ps = ps_feat.tile([P, 1], F32, tag="ddq")
                nc.tensor.matmul(
                    out=bc_ps, lhsT=ones_row, rhs=kst, start=True, stop=True
                )
                expns = work.tile([P, 1], F32)
                nc.scalar.activation(out=expns, in_=bc_ps, func=ACTF.Exp, scale=-1.0)
                scale_k = work.tile([P, 1], F32)
                nc.vector.tensor_scalar_mul(out=scale_k, in0=expns, scalar1=RATIO)

                for i in range(nt):
                    n0 = i * P
                    pr = min(P, n_nodes - n0)
                    ksl = kbuf[:pr, i * M : (i + 1) * M]
                    nc.vector.tensor_scalar(
                        out=ksl, in0=ksl,
                        scalar1=scale_k[:pr], scalar2=RATIO * EPS,
                        op0=ALU.mult, op1=ALU.add,
                    )
                    nc.sync.dma_start(out=kp[n0 : n0 + pr, :], in_=ksl)

            # ================= Phase B: kv = k'^T x, k_sum =================
            kv_sb = singles.tile([M, bpc, C], F32)
            ksum_sb = singles.tile([M, 1], F32)
            with tc.tile_pool(name="ps_acc", bufs=1, space="PSUM") as ps_acc:
                kv_ps = ps_acc.tile([M, bpc, C], F32, tag="kv")
                ksum_ps = ps_acc.tile([M, 1], F32, tag="ksum")
                # absorb the final pass-2 DVE tick on PE so the kv matmuls
                # below only carry their x-DMA waits
                primeb_ps = ps_acc.tile([1, 1], F32, tag="primeb")
                lastpr = n_nodes - (nt - 1) * P
                lastc = (nt - 1) * M + M - 1
                nc.tensor.matmul(
                    out=primeb_ps, lhsT=kbuf[:lastpr, lastc : lastc + 1],
                    rhs=ones_col[:lastpr], start=True, stop=True,
                )
                for i in range(nt):
                    n0 = i * P
                    pr = min(P, n_nodes - n0)
                    xt = xin.tile([P, bpc, C], F32)
                    for b in range(bpc):
                        nc.sync.dma_start(
                            out=xt[:pr, b, :], in_=x2[b, n0 : n0 + pr, :]
                        )
                    ksl = kbuf[:pr, i * M : (i + 1) * M]
                    nc.tensor.matmul(
                        out=_flat(kv_ps), lhsT=ksl, rhs=_flat(xt[:pr]),
                        start=(i == 0), stop=(i == nt - 1),
                    )
                    nc.tensor.matmul(
                        out=ksum_ps, lhsT=ksl, rhs=ones_col[:pr],
                        start=(i == 0), stop=(i == nt - 1),
                    )
                nc.vector.tensor_copy(out=_flat(kv_sb), in_=_flat(kv_ps))
                nc.vector.tensor_copy(out=ksum_sb, in_=ksum_ps)

            # ================= Phase C: num, denom, out =================
            with tc.tile_pool(name="ps_out", bufs=2, space="PSUM") as ps_out:
                for i in range(nt):
                    n0 = i * P
                    pr = min(P, n_nodes - n0)
                    qsl = qT_buf[:, n0 : n0 + pr]
                    num_ps = ps_out.tile([P, bpc, C], F32, tag="num")
                    nc.tensor.matmul(
                        out=_flat(num_ps[:pr]), lhsT=qsl, rhs=_flat(kv_sb),
                        start=True, stop=True,
                    )
                    den_ps = ps_out.tile([P, 1], F32, tag="den")
                    nc.tensor.matmul(
                        out=den_ps[:pr], lhsT=qsl, rhs=ksum_sb,
                        start=True, stop=True,
                    )
                    rden = work.tile([P, 1], F32)
                    nc.vector.reciprocal(out=rden[:pr], in_=den_ps[:pr])
                    outt = outp.tile([P, bpc, C], F32)
                    nc.vector.tensor_scalar(
                        out=_flat(outt[:pr]), in0=_flat(num_ps[:pr]),
                        scalar1=rden[:pr], scalar2=None, op0=ALU.mult,
                    )
                    for b in range(bpc):
                        nc.sync.dma_start(
                            out=out2[b, n0 : n0 + pr, :], in_=outt[:pr, b, :]
                        )

    nc.finalize()
    return nc


_PROG = None


def _get_prog():
    global _PROG
    if _PROG is None:
        _PROG = build()
    return _PROG


def run_on_hw(x, node_vec1, node_vec2, random_matrix, trace=False):
    from concourse.bass_utils import run_bass_kernel_spmd

    x = np.ascontiguousarray(np.asarray(x, dtype=np.float32))
    nv1 = np.ascontiguousarray(np.asarray(node_vec1, dtype=np.float32))
    nv2 = np.ascontiguousarray(np.asarray(node_vec2, dtype=np.float32))
    proj = np.ascontiguousarray(np.asarray(random_matrix, dtype=np.float32))

    nc = _get_prog()
    in_maps = [
        {"x2": x[i * BPC : (i + 1) * BPC], "nv1": nv1, "nv2": nv2, "proj": proj}
        for i in range(NCORES)
    ]
    res = run_bass_kernel_spmd(nc, in_maps, list(range(NCORES)), trace=trace)
    results = res.results
    out = np.concatenate([results[i]["out2"] for i in range(NCORES)], axis=0)
    return (out, results[0]["qp"], results[0]["kp"]), res


def kernel(x, node_vec1, node_vec2, random_matrix):
    (out, qprime, kprime), _ = run_on_hw(x, node_vec1, node_vec2, random_matrix)
    return out, qprime, kprime
